# revision 37
# baseline (speedup 1.0000x reference)
"""MultiHeadCrossAttention on 8 TRN2 NeuronCores.

Sharding: core c -> batch b = c//2, head-group g = c%2 (8 heads, 512 out dims).
Each core computes its head-group's Q/K/V projections, attention, and a
partial out-projection (Wo columns restricted to its head-group). Host sums
the two partials per batch and adds bo.

v2 schedule ("early-exp"):
  The ACT engine's exp stream (16.8M exps = ~147us) is the scarcest
  resource, so it starts as early as possible (~11us, vs 56us in v1):
  DMA order is wq, qT[:, :512], wk, kT[:, :512] first; the prologue is
  HAM-warmup matmuls + a dummy exp (ACT table preload) + Q(m0,n0) +
  K(m0,c0); attention then starts while the rest of the inputs stream in.
  The V projection runs inside the attention phase's spare PE slots.

  Iteration = (head-pair m, query-half n), n-outer: 8 iters x 16 key-chunk
  steps. A step issues a row-packed score PAIR -- heads 2m / 2m+1 live in
  partitions 0:64 / 64:128 of the qt/kt tiles, so two K=64 matmuls with
  tile_position (0,0) and (64,0) execute concurrently on the PE array --
  into one [128,1024] PSUM tile, then one exp covers both heads.

  PV (M=65, ones-column denominator like v1) runs statically ONE iteration
  behind scores/exp through a deep p2 pool, so V-projection feed work in
  the early iterations never stalls the exp stream. Normalization is per
  iteration: raw-denominator PE broadcast matmul, reciprocal, multiply.

  Projections (K m0c1..3, all K m1-3, Q, V, first-half out-proj) are fed
  into spare PE slots between score pairs via a budgeted pump, ordered by
  first use. Out-proj second half + PV of the last iter run in the tail.
"""

import contextlib
import sys

import numpy as np

if "/opt/trn_rl_repo" not in sys.path:
    sys.path.insert(0, "/opt/trn_rl_repo")

import concourse.bacc as bacc
import concourse.bass as bass
import concourse.mybir as mybir
import concourse.tile as tile
from concourse.bass_utils import run_bass_kernel_spmd

FP32 = mybir.dt.float32
FP16 = mybir.dt.float16

B, NQ, NK = 4, 1024, 2048
QD, KD = 1024, 768
H, D = 16, 64
E = H * D  # 1024 total embed dim
G = 8  # heads per core
GO = G * D  # 512 out dims per core
DA = D + 1  # 65: head dim + denominator column
SCALE = 1.0 / 8.0

# test.py hooks
TRACE = False
TRACE_KWARGS = {}
LAST_RESULT = None


def _mm(nc, out, lhsT, rhs, start, stop):
    nc.tensor.matmul(out, lhsT, rhs, start=start, stop=stop)


def build_program():
    nc = bacc.Bacc()

    qT = nc.declare_dram_parameter("qT", [QD, NQ], FP16, isOutput=False)
    kT = nc.declare_dram_parameter("kT", [KD, NK], FP16, isOutput=False)
    vT = nc.declare_dram_parameter("vT", [KD, NK], FP16, isOutput=False)
    wq = nc.declare_dram_parameter("wq", [QD, GO], FP16, isOutput=False)
    wk = nc.declare_dram_parameter("wk", [KD, GO], FP16, isOutput=False)
    wv = nc.declare_dram_parameter("wv", [KD, GO], FP16, isOutput=False)
    wo = nc.declare_dram_parameter("wo", [GO, E], FP16, isOutput=False)
    vbias = nc.declare_dram_parameter("vbias", [128, GO], FP32, isOutput=False)
    bq = nc.declare_dram_parameter("bq", [128, 4], FP32, isOutput=False)
    bk = nc.declare_dram_parameter("bk", [128, 4], FP32, isOutput=False)
    out = nc.declare_dram_parameter("out", [NQ, E], FP32, isOutput=True)

    with (
        nc.allow_low_precision("fp16 attention activations; validated vs oracle"),
        tile.TileContext(nc) as tc,
    ):
        with contextlib.ExitStack() as _st:

            def _pool(name, bufs=1, **kw):
                return _st.enter_context(tc.tile_pool(name=name, bufs=bufs, **kw))

            consts = _pool("consts")
            wo_p = _pool("wo_p")
            wq_p = _pool("wq_p")
            wk_p = _pool("wk_p")
            wv_p = _pool("wv_p")
            qin_p = _pool("qin_p")
            kin_p = _pool("kin_p")
            vin_p = _pool("vin_p")
            qt_p = _pool("qt_p")
            kt_p = _pool("kt_p")
            va_p = _pool("va_p")
            osb_p = _pool("osb_p")
            ys_p = _pool("ys_p", bufs=4)
            den_p = _pool("den_p", bufs=4)
            rc_p = _pool("rc_p", bufs=2)
            p2_p = _pool("p2_p", bufs=18)

            bq_sb = consts.tile([128, 4], FP32)
            bk_sb = consts.tile([128, 4], FP32)
            ones_sb = consts.tile([1, 64], FP16)
            nc.vector.memset(ones_sb[:], 1.0)
            warm_sb = consts.tile([128, 640], FP16)
            nc.vector.memset(warm_sb[:], 0.001)
            vbias_sb = consts.tile([128, 8, 64], FP32)

            wo_sb = wo_p.tile([128, 4, E], FP16, name="wo")
            wq_sb = wq_p.tile([128, 8, GO], FP16, name="wq")
            wk_sb = wk_p.tile([128, 6, GO], FP16, name="wk")
            wv_sb = wv_p.tile([128, 6, GO], FP16, name="wv")
            qin = qin_p.tile([128, 8, NQ], FP16, name="qin")
            kin = kin_p.tile([128, 6, NK], FP16, name="kin")
            vin = vin_p.tile([128, 6, NK], FP16, name="vin")

            # Persistent activation tiles.
            # qt[m][n]: Q proj dims-chunk m (= head pair 2m,2m+1 stacked in
            # partitions), query half n. kt[m][cc]: K proj, key chunk of 512.
            qt_sb = [
                [qt_p.tile([128, 512], FP16, name=f"qt{m}_{n}") for n in range(2)]
                for m in range(4)
            ]
            kt_sb = [
                [kt_p.tile([128, 512], FP16, name=f"kt{m}_{c}") for c in range(4)]
                for m in range(4)
            ]
            # V_aug[t]: [128 tokk, 8 heads, 65]; col 64 = ones (denominator)
            va_sb = [va_p.tile([128, 8, DA], FP16, name=f"va{t}") for t in range(16)]
            for t in range(16):
                nc.vector.memset(va_sb[t][:, :, 64:65], 1.0)
            # O^T (normalized) [concat dim 512 -> 4 tiles of 128, tokq 1024]
            osb = [osb_p.tile([128, NQ], FP16, name=f"osb{t}") for t in range(4)]

            # ---- DMA prefetch, ordered by first use. Score path (wq, qT
            # first half, wk, kT chunks) goes first so exp starts ~11us in.
            def blk(dram, g, t):
                return dram[:, :].rearrange("(g p) t -> p g t", p=128)

            # tiny bias DMAs first: DMA completion semaphores share lanes,
            # so a small late DMA can falsely gate an earlier big one's
            # consumers
            # The prologue's K(0,0)/Q(0,0) only read the m=0 column slice
            # of wk/wq, so those slices ship first: the critical chain to
            # the first exp is ~2.2MB instead of 3.5MB.
            nc.sync.dma_start(bq_sb[:], bq[:, :])
            nc.sync.dma_start(bk_sb[:], bk[:, :])
            nc.sync.dma_start(
                wk_sb[:, :, 0:128],
                wk[:, 0:128].rearrange("(g p) t -> p g t", p=128),
            )
            nc.sync.dma_start(
                kin[:, :, 0:512],
                kT[:, 0:512].rearrange("(g p) t -> p g t", p=128),
            )
            nc.sync.dma_start(
                wq_sb[:, :, 0:128],
                wq[:, 0:128].rearrange("(g p) t -> p g t", p=128),
            )
            nc.sync.dma_start(
                qin[:, :, 0:512],
                qT[:, 0:512].rearrange("(g p) t -> p g t", p=128),
            )
            nc.sync.dma_start(
                kin[:, :, 512:1024],
                kT[:, 512:1024].rearrange("(g p) t -> p g t", p=128),
            )
            nc.sync.dma_start(
                wk_sb[:, :, 128:512],
                wk[:, 128:512].rearrange("(g p) t -> p g t", p=128),
            )
            nc.sync.dma_start(
                wq_sb[:, :, 128:512],
                wq[:, 128:512].rearrange("(g p) t -> p g t", p=128),
            )
            nc.sync.dma_start(
                kin[:, :, 1024:2048],
                kT[:, 1024:2048].rearrange("(g p) t -> p g t", p=128),
            )
            nc.sync.dma_start(wv_sb[:, :, :], blk(wv, 6, GO))
            nc.sync.dma_start(vbias_sb[:, :, :], vbias[:, :])
            nc.sync.dma_start(
                vin[:, :, 0:1024],
                vT[:, 0:1024].rearrange("(g p) t -> p g t", p=128),
            )
            nc.sync.dma_start(
                vin[:, :, 1024:2048],
                vT[:, 1024:2048].rearrange("(g p) t -> p g t", p=128),
            )
            nc.sync.dma_start(
                qin[:, :, 512:1024],
                qT[:, 512:1024].rearrange("(g p) t -> p g t", p=128),
            )
            nc.sync.dma_start(wo_sb[:, :, :], blk(wo, 4, E))

            with contextlib.ExitStack() as _est:
                otp = _est.enter_context(tc.tile_pool(name="otp", bufs=2, space="PSUM"))
                stp = _est.enter_context(tc.tile_pool(name="stp", bufs=2, space="PSUM"))
                pfeed = _est.enter_context(
                    tc.tile_pool(name="pfeed", bufs=2, space="PSUM")
                )

                # ---- HAM warmup + ACT exp-table preload during DMA wait ----
                wps = pfeed.tile([128, 512], FP32, name="pfq")
                for i in range(8):
                    _mm(
                        nc,
                        wps[:],
                        warm_sb[:, 0:128],
                        warm_sb[:, 128:640],
                        start=True,
                        stop=True,
                    )
                warm_act = den_p.tile([1, 64], FP16, name="wact")
                nc.scalar.activation(
                    warm_act[:],
                    ones_sb[:],
                    mybir.ActivationFunctionType.Exp,
                    bias=0.0,
                    scale=1.0,
                )

                # ---- Projection chunk emitters ----
                def q_chunk(m, n):
                    psq = pfeed.tile([128, 512], FP32, name="pfq")
                    for kk in range(8):
                        _mm(
                            nc,
                            psq[:],
                            wq_sb[:, kk, m * 128 : (m + 1) * 128],
                            qin[:, kk, n * 512 : (n + 1) * 512],
                            start=(kk == 0),
                            stop=(kk == 7),
                        )
                    nc.vector.tensor_scalar_add(
                        qt_sb[m][n][:], psq[:], bq_sb[:, m : m + 1]
                    )

                def k_chunk(m, c):
                    psk = pfeed.tile([128, 512], FP32, name="pfq")
                    for kk in range(6):
                        _mm(
                            nc,
                            psk[:],
                            wk_sb[:, kk, m * 128 : (m + 1) * 128],
                            kin[:, kk, c * 512 : (c + 1) * 512],
                            start=(kk == 0),
                            stop=(kk == 5),
                        )
                    nc.vector.tensor_scalar_add(
                        kt_sb[m][c][:], psk[:], bk_sb[:, m : m + 1]
                    )

                # ---- Feed: single-matmul (or small-group) emitters, popped
                # between score steps, ordered by first-use deadline:
                #   K(0,c1-3), K(1,*), Q(1,0)   <- iter 1 = (m1, n0)
                #   V tb0, tb1                  <- PV of iter 0 (lag-1, in it1)
                #   K(2,*), Q(2,0), V tb2       <- iter 2 / PV of it1
                #   K(3,*), Q(3,0), V tb3       <- iter 3 / PV of it2
                #   Q(m,1) x4                   <- iters 4-7
                #   out-proj n=0 half           <- appended at iter 5 (gated
                #                                  on norm(3,0) emitted it4)
                # ordered by DMA arrival (kin c0/c1, then kin c2/c3, then
                # vin halves) AND consumption deadline (K/Q of pair m by
                # iter m; va[t] progressively during iter 1)
                feed = [
                    ("k", 0, 1), ("k", 1, 0), ("k", 1, 1), ("q", 1, 0),
                    ("k", 0, 2), ("k", 0, 3), ("k", 1, 2), ("k", 1, 3),
                ]
                for tb in range(2):
                    for t2 in range(4):
                        feed.append(("v", tb, t2))
                feed += [("k", 2, 0), ("k", 2, 1), ("q", 2, 0)]
                for t2 in range(4):
                    feed.append(("v", 2, t2))
                feed += [("k", 2, 2), ("k", 2, 3)]
                for t2 in range(4):
                    feed.append(("v", 3, t2))
                feed += [("k", 3, 0), ("k", 3, 1), ("k", 3, 2), ("k", 3, 3),
                         ("q", 3, 0)]
                for m in range(4):
                    feed.append(("q", m, 1))

                kdone = [[False] * 4 for _ in range(4)]
                kdone[0][0] = True  # prologue
                qdone = [[False, False] for _ in range(4)]
                qdone[0][0] = True  # prologue
                vadone = [False] * 16

                # Chunk-atomic pump: each item emits a complete accumulation
                # group (psum alloc -> MMs -> bias add), so at any pump
                # boundary at most one pfeed buffer has a pending read and
                # the 2-buffer rotation can never deadlock.
                def pump(budget):
                    while budget > 0 and feed:
                        item = feed.pop(0)
                        if item[0] == "q":
                            _, m, n = item
                            psq = pfeed.tile([128, 512], FP32, name="pfq")
                            for kk in range(8):
                                _mm(
                                    nc,
                                    psq[:],
                                    wq_sb[:, kk, m * 128 : (m + 1) * 128],
                                    qin[:, kk, n * 512 : (n + 1) * 512],
                                    start=(kk == 0),
                                    stop=(kk == 7),
                                )
                            nc.vector.tensor_scalar_add(
                                qt_sb[m][n][:], psq[:], bq_sb[:, m : m + 1]
                            )
                            qdone[m][n] = True
                            budget -= 8
                        elif item[0] == "k":
                            _, m, c = item
                            psk = pfeed.tile([128, 512], FP32, name="pfq")
                            for kk in range(6):
                                _mm(
                                    nc,
                                    psk[:],
                                    wk_sb[:, kk, m * 128 : (m + 1) * 128],
                                    kin[:, kk, c * 512 : (c + 1) * 512],
                                    start=(kk == 0),
                                    stop=(kk == 5),
                                )
                            nc.vector.tensor_scalar_add(
                                kt_sb[m][c][:], psk[:], bk_sb[:, m : m + 1]
                            )
                            kdone[m][c] = True
                            budget -= 6
                        elif item[0] == "v":
                            _, tb, t2 = item
                            psv = pfeed.tile([128, 8, 64], FP32, name="pfq")
                            c0 = tb * 512 + t2 * 128
                            for kk in range(6):
                                _mm(
                                    nc,
                                    psv[:, :, :],
                                    vin[:, kk, c0 : c0 + 128],
                                    wv_sb[:, kk, :],
                                    start=(kk == 0),
                                    stop=(kk == 5),
                                )
                            nc.vector.tensor_add(
                                va_sb[tb * 4 + t2][:, :, 0:64],
                                psv[:, :, :],
                                vbias_sb[:, :, :],
                            )
                            vadone[tb * 4 + t2] = True
                            budget -= 6
                        else:  # out-proj chunk: 4 mms + copy + dma
                            _, mo, no = item
                            psy = pfeed.tile([128, 512], FP32, name="pfq")
                            for kt4 in range(4):
                                _mm(
                                    nc,
                                    psy[:],
                                    osb[kt4][:, mo * 128 : (mo + 1) * 128],
                                    wo_sb[:, kt4, no * 512 : (no + 1) * 512],
                                    start=(kt4 == 0),
                                    stop=(kt4 == 3),
                                )
                            ys = ys_p.tile([128, 512], FP32, name="ys")
                            tailmode = mo >= 4  # ACT idle, spread engines
                            if tailmode and no == 0:
                                nc.scalar.activation(
                                    ys[:], psy[:],
                                    mybir.ActivationFunctionType.Copy,
                                )
                            else:
                                nc.vector.tensor_copy(ys[:], psy[:])
                            dma_eng = (
                                nc.scalar if (tailmode and no == 1) else nc.sync
                            )
                            dma_eng.dma_start(
                                out[
                                    mo * 128 : (mo + 1) * 128,
                                    no * 512 : (no + 1) * 512,
                                ],
                                ys[:],
                            )
                            budget -= 4

                # ---- Prologue: K(0,0) first (its DMA lands first; the
                # warmup matmuls flow straight into it, keeping HAM warm),
                # then Q(0,0).
                k_chunk(0, 0)
                q_chunk(0, 0)

                # ---- Main loop: 8 iters x 16 steps ----
                iters = [(m, n) for n in range(2) for m in range(4)]
                # p2 tiles of the previous iter, for lag-1 PV
                p2_prev = None
                ot_prev = None  # (ot_A, ot_B, m_prev, n_prev)

                def emit_pv(ot_pair, mp, c, p2t):
                    ot_A, ot_B = ot_pair
                    _mm(
                        nc,
                        ot_A[:65, :],
                        va_sb[c][:, 2 * mp : 2 * mp + 1, :],
                        p2t[:, 0:512],
                        start=(c == 0),
                        stop=(c == 15),
                    )
                    _mm(
                        nc,
                        ot_B[:65, :],
                        va_sb[c][:, 2 * mp + 1 : 2 * mp + 2, :],
                        p2t[:, 512:1024],
                        start=(c == 0),
                        stop=(c == 15),
                    )

                def emit_norm(ot_pair, mp, np_, use_act=False, bc_pool=None):
                    # normalize both heads of the pair into osb; one pfeed
                    # bc tile reused for both heads (A's recip must drain
                    # before B's broadcast overwrites -- DVE keeps up).
                    # use_act: offload den copies to the (idle) scalar
                    # engine in the tail to shorten the final norm latency.
                    if bc_pool is None:
                        bc_t = pfeed.tile([128, 512], FP32, name="pfq")
                    else:
                        bc_t = bc_pool.tile([128, 1024], FP32, name="st")[:, 0:512]
                    for hl, ot in enumerate(ot_pair):
                        den_t = den_p.tile([1, 512], FP16, name="den")
                        if use_act:
                            nc.scalar.activation(
                                den_t[:], ot[64:65, :],
                                mybir.ActivationFunctionType.Copy,
                            )
                        else:
                            nc.vector.tensor_copy(den_t[:], ot[64:65, :])
                        nc.tensor.matmul(
                            bc_t[:64, :], ones_sb[:, :], den_t[:],
                            start=True, stop=True,
                        )
                        rc_t = rc_p.tile([64, 512], FP32, name="rc")
                        nc.vector.reciprocal_approx_fast(rc_t[:], bc_t[:64, :])
                        nc.vector.tensor_mul(
                            osb[mp][hl * 64 : (hl + 1) * 64,
                                    np_ * 512 : (np_ + 1) * 512],
                            ot[:64, :],
                            rc_t[:],
                        )

                # per-step feed budget in MM units (chunk pops overdraw by
                # design; the while-safeguards below guarantee deadlines)
                budgets = [6, 6, 5, 4, 3, 3, 2, 2]
                for it in range(8):
                    m, n = iters[it]
                    if it == 5:
                        # norm(3,0) was emitted during iter 4 -> the n=0
                        # half of osb is fully normalized in program order
                        for mo in range(4):
                            for no in range(2):
                                feed.append(("f", mo, no))
                    # safeguard: the iter's first scores need qt[m][n]
                    while feed and not qdone[m][n]:
                        pump(6)
                    ot_cur = (
                        otp.tile([128, 512], FP32, name="ot"),
                        otp.tile([128, 512], FP32, name="ot"),
                    )
                    p2_cur = []

                    def emit_score(c):
                        # one row-packed score pair into a fresh st tile
                        cc, co = c // 4, (c % 4) * 128
                        while feed and not kdone[m][cc]:
                            pump(6)
                        st = stp.tile([128, 1024], FP32, name="st")
                        _mm(
                            nc,
                            st[:, 0:512],
                            kt_sb[m][cc][0:64, co : co + 128],
                            qt_sb[m][n][0:64, :],
                            start=True,
                            stop=True,
                        )
                        _mm(
                            nc,
                            st[:, 512:1024],
                            kt_sb[m][cc][64:128, co : co + 128],
                            qt_sb[m][n][64:128, :],
                            start=True,
                            stop=True,
                        )
                        return st

                    # Steps emitted in PAIRS: the two score pairs go
                    # back-to-back so the second pair's LDWEIGHTS (rows
                    # 0:64) pulls ahead under the first pair's in-flight
                    # row-disjoint matmuls instead of waiting behind a
                    # full-row PV/feed matmul.
                    for c2 in range(8):
                        ca, cb = 2 * c2, 2 * c2 + 1
                        st_a = emit_score(ca)
                        st_b = emit_score(cb)
                        for c, st in ((ca, st_a), (cb, st_b)):
                            p2t = p2_p.tile([128, 1024], FP16, name="p2")
                            if it >= 3 and c in (3, 7, 11):
                                # Integer-trick exp on the vector engine
                                # (Schraudolph in fp16 bit space): bits =
                                # round(1024*(log2e*scale*s + 15 + C)),
                                # C = -0.0434 minimizes max rel err (~3%).
                                # Offloads 3/16 chunks of the ACT-bound
                                # iters 3-7; validated end-to-end 4.6e-3.
                                nc.vector.tensor_scalar(
                                    p2t[:].bitcast(mybir.dt.uint16),
                                    st[:],
                                    184.664965,
                                    15315.5584,
                                    mybir.AluOpType.mult,
                                    mybir.AluOpType.add,
                                )
                            else:
                                nc.scalar.activation(
                                    p2t[:],
                                    st[:],
                                    mybir.ActivationFunctionType.Exp,
                                    bias=0.0,
                                    scale=SCALE,
                                )
                            p2_cur.append(p2t)
                        # lag-1 PV of the previous iteration
                        if p2_prev is not None:
                            mp, np_ = iters[it - 1]
                            for c in (ca, cb):
                                while feed and not vadone[c]:
                                    pump(6)
                                emit_pv(ot_prev, mp, c, p2_prev[c])
                            if cb == 15:
                                emit_norm(ot_prev, mp, np_)
                        pump(2 * budgets[it])
                    p2_prev, ot_prev = p2_cur, ot_cur

                # ---- Tail: PV of the last iter, then out-proj second half.
                # The scores/exps are done, so the stp banks host FOUR
                # pre-started psy accumulators: their kt4=0..2 partial sums
                # (osb[0..2] normalized during iters 5-7) run concurrently
                # with the final normalize chain; only each chunk's kt4=3
                # matmul waits for norm(3,1).
                mp, np_ = iters[7]
                for c in range(16):
                    emit_pv(ot_prev, mp, c, p2_prev[c])
                pre = []  # (psy_ap, mo, no)
                st_t1 = stp.tile([128, 1024], FP32, name="st")
                st_t2 = stp.tile([128, 1024], FP32, name="st")
                for i, (mo, no) in enumerate(
                    [(4, 0), (4, 1), (5, 0), (5, 1)]
                ):
                    t = st_t1 if i < 2 else st_t2
                    psy = t[:, (i % 2) * 512 : (i % 2 + 1) * 512]
                    pre.append((psy, mo, no))
                    for kt4 in range(3):
                        _mm(
                            nc,
                            psy,
                            osb[kt4][:, mo * 128 : (mo + 1) * 128],
                            wo_sb[:, kt4, no * 512 : (no + 1) * 512],
                            start=(kt4 == 0),
                            stop=False,
                        )
                emit_norm(ot_prev, mp, np_, use_act=True)
                for i, (psy, mo, no) in enumerate(pre):
                    _mm(
                        nc,
                        psy,
                        osb[3][:, mo * 128 : (mo + 1) * 128],
                        wo_sb[:, 3, no * 512 : (no + 1) * 512],
                        start=False,
                        stop=True,
                    )
                    ys = ys_p.tile([128, 512], FP32, name="ys")
                    if no == 0:
                        nc.scalar.activation(
                            ys[:], psy,
                            mybir.ActivationFunctionType.Copy,
                        )
                    else:
                        nc.vector.tensor_copy(ys[:], psy)
                    (nc.scalar if no == 1 else nc.sync).dma_start(
                        out[mo * 128 : (mo + 1) * 128, no * 512 : (no + 1) * 512],
                        ys[:],
                    )
                for mo in range(6, 8):
                    for no in range(2):
                        feed.append(("f", mo, no))
                pump(10**9)

    nc.finalize()
    return nc


def kernel(**inputs):
    global LAST_RESULT
    arrs = {k: np.asarray(v, dtype=np.float32) for k, v in inputs.items()}
    query, key, value = arrs["query"], arrs["key"], arrs["value"]
    Wq, bq_, Wk, bk_ = arrs["Wq"], arrs["bq"], arrs["Wk"], arrs["bk"]
    Wv, bv_, Wo, bo_ = arrs["Wv"], arrs["bv"], arrs["Wo"], arrs["bo"]

    nc = build_program()

    qTb = [np.ascontiguousarray(query[b].T.astype(np.float16)) for b in range(B)]
    kTb = [np.ascontiguousarray(key[b].T.astype(np.float16)) for b in range(B)]
    vTb = [np.ascontiguousarray(value[b].T.astype(np.float16)) for b in range(B)]

    per_group = []
    for g in range(2):
        gs = slice(g * GO, (g + 1) * GO)
        wq_m = np.ascontiguousarray(Wq[gs, :].T.astype(np.float16))
        wk_m = np.ascontiguousarray(Wk[gs, :].T.astype(np.float16))
        wv_m = np.ascontiguousarray(Wv[gs, :].T.astype(np.float16))
        vb_row = bv_[gs].astype(np.float32)  # head-major [8*64]
        vbias_m = np.ascontiguousarray(np.tile(vb_row, (128, 1)).astype(np.float32))
        wo_m = np.ascontiguousarray(Wo[:, gs].T.astype(np.float16))
        bq_m = np.ascontiguousarray(bq_[gs].reshape(4, 128).T)
        bk_m = np.ascontiguousarray(bk_[gs].reshape(4, 128).T)
        per_group.append(
            {
                "wq": wq_m,
                "wk": wk_m,
                "wv": wv_m,
                "wo": wo_m,
                "vbias": vbias_m,
                "bq": bq_m,
                "bk": bk_m,
            }
        )

    in_maps = []
    for c in range(8):
        b, g = c // 2, c % 2
        m = {"qT": qTb[b], "kT": kTb[b], "vT": vTb[b]}
        m.update(per_group[g])
        in_maps.append(m)

    res = run_bass_kernel_spmd(
        nc, in_maps, list(range(8)), trace=TRACE, **(TRACE_KWARGS if TRACE else {})
    )
    LAST_RESULT = res

    outs = res.results
    Y = np.empty((B, NQ, E), np.float32)
    for b in range(B):
        Y[b] = outs[2 * b]["out"] + outs[2 * b + 1]["out"] + bo_[None, :]
    return Y


# revision 38
# speedup vs baseline: 1.1034x; 1.1034x over previous
"""MultiHeadCrossAttention on 8 TRN2 NeuronCores.

Sharding: core c -> batch b = c//2, head-group g = c%2 (8 heads, 512 out dims).
Each core computes its head-group's Q/K/V projections, attention, and a
partial out-projection (Wo columns restricted to its head-group). Host sums
the two partials per batch and adds bo.

v2 schedule ("early-exp"):
  The ACT engine's exp stream (16.8M exps = ~147us) is the scarcest
  resource, so it starts as early as possible (~11us, vs 56us in v1):
  DMA order is wq, qT[:, :512], wk, kT[:, :512] first; the prologue is
  HAM-warmup matmuls + a dummy exp (ACT table preload) + Q(m0,n0) +
  K(m0,c0); attention then starts while the rest of the inputs stream in.
  The V projection runs inside the attention phase's spare PE slots.

  Iteration = (head-pair m, query-half n), n-outer: 8 iters x 16 key-chunk
  steps. A step issues a row-packed score PAIR -- heads 2m / 2m+1 live in
  partitions 0:64 / 64:128 of the qt/kt tiles, so two K=64 matmuls with
  tile_position (0,0) and (64,0) execute concurrently on the PE array --
  into one [128,1024] PSUM tile, then one exp covers both heads.

  PV (M=65, ones-column denominator like v1) runs statically ONE iteration
  behind scores/exp through a deep p2 pool, so V-projection feed work in
  the early iterations never stalls the exp stream. Normalization is per
  iteration: raw-denominator PE broadcast matmul, reciprocal, multiply.

  Projections (K m0c1..3, all K m1-3, Q, V, first-half out-proj) are fed
  into spare PE slots between score pairs via a budgeted pump, ordered by
  first use. Out-proj second half + PV of the last iter run in the tail.
"""

import contextlib
import sys

import numpy as np

if "/opt/trn_rl_repo" not in sys.path:
    sys.path.insert(0, "/opt/trn_rl_repo")

import concourse.bacc as bacc
import concourse.bass as bass
import concourse.mybir as mybir
import concourse.tile as tile
from concourse.bass_utils import run_bass_kernel_spmd

FP32 = mybir.dt.float32
FP16 = mybir.dt.float16

B, NQ, NK = 4, 1024, 2048
QD, KD = 1024, 768
H, D = 16, 64
E = H * D  # 1024 total embed dim
G = 8  # heads per core
GO = G * D  # 512 out dims per core
DA = D + 1  # 65: head dim + denominator column
SCALE = 1.0 / 8.0

# test.py hooks
TRACE = False
TRACE_KWARGS = {}
LAST_RESULT = None


def _mm(nc, out, lhsT, rhs, start, stop):
    nc.tensor.matmul(out, lhsT, rhs, start=start, stop=stop)


def build_program():
    nc = bacc.Bacc()

    qT = nc.declare_dram_parameter("qT", [QD, NQ], FP16, isOutput=False)
    kT = nc.declare_dram_parameter("kT", [KD, NK], FP16, isOutput=False)
    vT = nc.declare_dram_parameter("vT", [KD, NK], FP16, isOutput=False)
    wq = nc.declare_dram_parameter("wq", [QD, GO], FP16, isOutput=False)
    wk = nc.declare_dram_parameter("wk", [KD, GO], FP16, isOutput=False)
    wv = nc.declare_dram_parameter("wv", [KD, GO], FP16, isOutput=False)
    wo = nc.declare_dram_parameter("wo", [GO, E], FP16, isOutput=False)
    vbias = nc.declare_dram_parameter("vbias", [128, GO], FP32, isOutput=False)
    bq = nc.declare_dram_parameter("bq", [128, 4], FP32, isOutput=False)
    bk = nc.declare_dram_parameter("bk", [128, 4], FP32, isOutput=False)
    out = nc.declare_dram_parameter("out", [NQ, E], FP32, isOutput=True)

    with (
        nc.allow_low_precision("fp16 attention activations; validated vs oracle"),
        tile.TileContext(nc) as tc,
    ):
        with contextlib.ExitStack() as _st:

            def _pool(name, bufs=1, **kw):
                return _st.enter_context(tc.tile_pool(name=name, bufs=bufs, **kw))

            consts = _pool("consts")
            wo_p = _pool("wo_p")
            wq_p = _pool("wq_p")
            wk_p = _pool("wk_p")
            wv_p = _pool("wv_p")
            qin_p = _pool("qin_p")
            kin_p = _pool("kin_p")
            vin_p = _pool("vin_p")
            qt_p = _pool("qt_p")
            kt_p = _pool("kt_p")
            va_p = _pool("va_p")
            osb_p = _pool("osb_p")
            ys_p = _pool("ys_p", bufs=4)
            den_p = _pool("den_p", bufs=4)
            rc_p = _pool("rc_p", bufs=2)
            p2_p = _pool("p2_p", bufs=18)

            bq_sb = consts.tile([128, 4], FP32)
            bk_sb = consts.tile([128, 4], FP32)
            ones_sb = consts.tile([1, 64], FP16)
            nc.vector.memset(ones_sb[:], 1.0)
            warm_sb = consts.tile([128, 640], FP16)
            nc.vector.memset(warm_sb[:], 0.001)
            vbias_sb = consts.tile([128, 8, 64], FP32)

            wo_sb = wo_p.tile([128, 4, E], FP16, name="wo")
            wq_sb = wq_p.tile([128, 8, GO], FP16, name="wq")
            wk_sb = wk_p.tile([128, 6, GO], FP16, name="wk")
            wv_sb = wv_p.tile([128, 6, GO], FP16, name="wv")
            qin = qin_p.tile([128, 8, NQ], FP16, name="qin")
            kin = kin_p.tile([128, 6, NK], FP16, name="kin")
            vin = vin_p.tile([128, 6, NK], FP16, name="vin")

            # Persistent activation tiles.
            # qt[m][n]: Q proj dims-chunk m (= head pair 2m,2m+1 stacked in
            # partitions), query half n. kt[m][cc]: K proj, key chunk of 512.
            qt_sb = [
                [qt_p.tile([128, 512], FP16, name=f"qt{m}_{n}") for n in range(2)]
                for m in range(4)
            ]
            kt_sb = [
                [kt_p.tile([128, 512], FP16, name=f"kt{m}_{c}") for c in range(4)]
                for m in range(4)
            ]
            # V_aug[t]: [128 tokk, 8 heads, 65]; col 64 = ones (denominator)
            va_sb = [va_p.tile([128, 8, DA], FP16, name=f"va{t}") for t in range(16)]
            for t in range(16):
                nc.vector.memset(va_sb[t][:, :, 64:65], 1.0)
            # O^T (normalized) [concat dim 512 -> 4 tiles of 128, tokq 1024]
            osb = [osb_p.tile([128, NQ], FP16, name=f"osb{t}") for t in range(4)]

            # ---- DMA prefetch, ordered by first use. Score path (wq, qT
            # first half, wk, kT chunks) goes first so exp starts ~11us in.
            def blk(dram, g, t):
                return dram[:, :].rearrange("(g p) t -> p g t", p=128)

            # tiny bias DMAs first: DMA completion semaphores share lanes,
            # so a small late DMA can falsely gate an earlier big one's
            # consumers
            # The prologue's K(0,0)/Q(0,0) only read the m=0 column slice
            # of wk/wq, so those slices ship first: the critical chain to
            # the first exp is ~2.2MB instead of 3.5MB.
            nc.sync.dma_start(bq_sb[:], bq[:, :])
            nc.sync.dma_start(bk_sb[:], bk[:, :])
            nc.sync.dma_start(
                wk_sb[:, :, 0:128],
                wk[:, 0:128].rearrange("(g p) t -> p g t", p=128),
            )
            nc.sync.dma_start(
                kin[:, :, 0:512],
                kT[:, 0:512].rearrange("(g p) t -> p g t", p=128),
            )
            nc.sync.dma_start(
                wq_sb[:, :, 0:128],
                wq[:, 0:128].rearrange("(g p) t -> p g t", p=128),
            )
            nc.sync.dma_start(
                qin[:, :, 0:512],
                qT[:, 0:512].rearrange("(g p) t -> p g t", p=128),
            )
            nc.sync.dma_start(
                kin[:, :, 512:1024],
                kT[:, 512:1024].rearrange("(g p) t -> p g t", p=128),
            )
            nc.sync.dma_start(
                wk_sb[:, :, 128:512],
                wk[:, 128:512].rearrange("(g p) t -> p g t", p=128),
            )
            nc.sync.dma_start(
                wq_sb[:, :, 128:512],
                wq[:, 128:512].rearrange("(g p) t -> p g t", p=128),
            )
            nc.sync.dma_start(
                kin[:, :, 1024:2048],
                kT[:, 1024:2048].rearrange("(g p) t -> p g t", p=128),
            )
            nc.sync.dma_start(wv_sb[:, :, :], blk(wv, 6, GO))
            nc.sync.dma_start(vbias_sb[:, :, :], vbias[:, :])
            nc.sync.dma_start(
                vin[:, :, 0:1024],
                vT[:, 0:1024].rearrange("(g p) t -> p g t", p=128),
            )
            nc.sync.dma_start(
                vin[:, :, 1024:2048],
                vT[:, 1024:2048].rearrange("(g p) t -> p g t", p=128),
            )
            nc.sync.dma_start(
                qin[:, :, 512:1024],
                qT[:, 512:1024].rearrange("(g p) t -> p g t", p=128),
            )
            nc.sync.dma_start(wo_sb[:, :, :], blk(wo, 4, E))

            with contextlib.ExitStack() as _est:
                otp = _est.enter_context(tc.tile_pool(name="otp", bufs=2, space="PSUM"))
                stp = _est.enter_context(tc.tile_pool(name="stp", bufs=2, space="PSUM"))
                pfeed = _est.enter_context(
                    tc.tile_pool(name="pfeed", bufs=2, space="PSUM")
                )

                # ---- HAM warmup + ACT exp-table preload during DMA wait ----
                wps = pfeed.tile([128, 512], FP32, name="pfq")
                for i in range(8):
                    _mm(
                        nc,
                        wps[:],
                        warm_sb[:, 0:128],
                        warm_sb[:, 128:640],
                        start=True,
                        stop=True,
                    )
                warm_act = den_p.tile([1, 64], FP16, name="wact")
                nc.scalar.activation(
                    warm_act[:],
                    ones_sb[:],
                    mybir.ActivationFunctionType.Exp,
                    bias=0.0,
                    scale=1.0,
                )

                # ---- Projection chunk emitters ----
                def q_chunk(m, n):
                    psq = pfeed.tile([128, 512], FP32, name="pfq")
                    for kk in range(8):
                        _mm(
                            nc,
                            psq[:],
                            wq_sb[:, kk, m * 128 : (m + 1) * 128],
                            qin[:, kk, n * 512 : (n + 1) * 512],
                            start=(kk == 0),
                            stop=(kk == 7),
                        )
                    nc.vector.tensor_scalar_add(
                        qt_sb[m][n][:], psq[:], bq_sb[:, m : m + 1]
                    )

                def k_chunk(m, c):
                    psk = pfeed.tile([128, 512], FP32, name="pfq")
                    for kk in range(6):
                        _mm(
                            nc,
                            psk[:],
                            wk_sb[:, kk, m * 128 : (m + 1) * 128],
                            kin[:, kk, c * 512 : (c + 1) * 512],
                            start=(kk == 0),
                            stop=(kk == 5),
                        )
                    nc.vector.tensor_scalar_add(
                        kt_sb[m][c][:], psk[:], bk_sb[:, m : m + 1]
                    )

                # ---- Feed: single-matmul (or small-group) emitters, popped
                # between score steps, ordered by first-use deadline:
                #   K(0,c1-3), K(1,*), Q(1,0)   <- iter 1 = (m1, n0)
                #   V tb0, tb1                  <- PV of iter 0 (lag-1, in it1)
                #   K(2,*), Q(2,0), V tb2       <- iter 2 / PV of it1
                #   K(3,*), Q(3,0), V tb3       <- iter 3 / PV of it2
                #   Q(m,1) x4                   <- iters 4-7
                #   out-proj n=0 half           <- appended at iter 5 (gated
                #                                  on norm(3,0) emitted it4)
                # ordered by DMA arrival (kin c0/c1, then kin c2/c3, then
                # vin halves) AND consumption deadline (K/Q of pair m by
                # iter m; va[t] progressively during iter 1)
                feed = [
                    ("k", 0, 1), ("k", 1, 0), ("k", 1, 1), ("q", 1, 0),
                    ("k", 0, 2), ("k", 0, 3), ("k", 1, 2), ("k", 1, 3),
                ]
                for tb in range(2):
                    for t2 in range(4):
                        feed.append(("v", tb, t2))
                feed += [("k", 2, 0), ("k", 2, 1), ("q", 2, 0)]
                for t2 in range(4):
                    feed.append(("v", 2, t2))
                feed += [("k", 2, 2), ("k", 2, 3)]
                for t2 in range(4):
                    feed.append(("v", 3, t2))
                feed += [("k", 3, 0), ("k", 3, 1), ("k", 3, 2), ("k", 3, 3),
                         ("q", 3, 0)]
                for m in range(4):
                    feed.append(("q", m, 1))

                kdone = [[False] * 4 for _ in range(4)]
                kdone[0][0] = True  # prologue
                qdone = [[False, False] for _ in range(4)]
                qdone[0][0] = True  # prologue
                vadone = [False] * 16

                # Chunk-atomic pump: each item emits a complete accumulation
                # group (psum alloc -> MMs -> bias add), so at any pump
                # boundary at most one pfeed buffer has a pending read and
                # the 2-buffer rotation can never deadlock.
                def pump(budget):
                    while budget > 0 and feed:
                        item = feed.pop(0)
                        if item[0] == "q":
                            _, m, n = item
                            psq = pfeed.tile([128, 512], FP32, name="pfq")
                            for kk in range(8):
                                _mm(
                                    nc,
                                    psq[:],
                                    wq_sb[:, kk, m * 128 : (m + 1) * 128],
                                    qin[:, kk, n * 512 : (n + 1) * 512],
                                    start=(kk == 0),
                                    stop=(kk == 7),
                                )
                            nc.vector.tensor_scalar_add(
                                qt_sb[m][n][:], psq[:], bq_sb[:, m : m + 1]
                            )
                            qdone[m][n] = True
                            budget -= 8
                        elif item[0] == "k":
                            _, m, c = item
                            psk = pfeed.tile([128, 512], FP32, name="pfq")
                            for kk in range(6):
                                _mm(
                                    nc,
                                    psk[:],
                                    wk_sb[:, kk, m * 128 : (m + 1) * 128],
                                    kin[:, kk, c * 512 : (c + 1) * 512],
                                    start=(kk == 0),
                                    stop=(kk == 5),
                                )
                            nc.vector.tensor_scalar_add(
                                kt_sb[m][c][:], psk[:], bk_sb[:, m : m + 1]
                            )
                            kdone[m][c] = True
                            budget -= 6
                        elif item[0] == "v":
                            _, tb, t2 = item
                            psv = pfeed.tile([128, 8, 64], FP32, name="pfq")
                            c0 = tb * 512 + t2 * 128
                            for kk in range(6):
                                _mm(
                                    nc,
                                    psv[:, :, :],
                                    vin[:, kk, c0 : c0 + 128],
                                    wv_sb[:, kk, :],
                                    start=(kk == 0),
                                    stop=(kk == 5),
                                )
                            nc.vector.tensor_add(
                                va_sb[tb * 4 + t2][:, :, 0:64],
                                psv[:, :, :],
                                vbias_sb[:, :, :],
                            )
                            vadone[tb * 4 + t2] = True
                            budget -= 6
                        else:  # out-proj chunk: 4 mms + copy + dma
                            _, mo, no = item
                            psy = pfeed.tile([128, 512], FP32, name="pfq")
                            for kt4 in range(4):
                                _mm(
                                    nc,
                                    psy[:],
                                    osb[kt4][:, mo * 128 : (mo + 1) * 128],
                                    wo_sb[:, kt4, no * 512 : (no + 1) * 512],
                                    start=(kt4 == 0),
                                    stop=(kt4 == 3),
                                )
                            ys = ys_p.tile([128, 512], FP32, name="ys")
                            tailmode = mo >= 4  # ACT idle, spread engines
                            if tailmode and no == 0:
                                nc.scalar.activation(
                                    ys[:], psy[:],
                                    mybir.ActivationFunctionType.Copy,
                                )
                            else:
                                nc.vector.tensor_copy(ys[:], psy[:])
                            dma_eng = (
                                nc.scalar if (tailmode and no == 1) else nc.sync
                            )
                            dma_eng.dma_start(
                                out[
                                    mo * 128 : (mo + 1) * 128,
                                    no * 512 : (no + 1) * 512,
                                ],
                                ys[:],
                            )
                            budget -= 4

                # ---- Prologue: K(0,0) first (its DMA lands first; the
                # warmup matmuls flow straight into it, keeping HAM warm),
                # then Q(0,0).
                k_chunk(0, 0)
                q_chunk(0, 0)

                # ---- Main loop: 8 iters x 16 steps ----
                iters = [(m, n) for n in range(2) for m in range(4)]
                # p2 tiles of the previous iter, for lag-1 PV
                p2_prev = None
                ot_prev = None  # (ot_A, ot_B, m_prev, n_prev)

                def emit_pv(ot_pair, mp, c, p2t):
                    ot_A, ot_B = ot_pair
                    _mm(
                        nc,
                        ot_A[:65, :],
                        va_sb[c][:, 2 * mp : 2 * mp + 1, :],
                        p2t[:, 0:512],
                        start=(c == 0),
                        stop=(c == 15),
                    )
                    _mm(
                        nc,
                        ot_B[:65, :],
                        va_sb[c][:, 2 * mp + 1 : 2 * mp + 2, :],
                        p2t[:, 512:1024],
                        start=(c == 0),
                        stop=(c == 15),
                    )

                def emit_norm(ot_pair, mp, np_, use_act=False, bc_pool=None):
                    # normalize both heads of the pair into osb; one pfeed
                    # bc tile reused for both heads (A's recip must drain
                    # before B's broadcast overwrites -- DVE keeps up).
                    # use_act: offload den copies to the (idle) scalar
                    # engine in the tail to shorten the final norm latency.
                    if bc_pool is None:
                        bc_t = pfeed.tile([128, 512], FP32, name="pfq")
                    else:
                        bc_t = bc_pool.tile([128, 1024], FP32, name="st")[:, 0:512]
                    for hl, ot in enumerate(ot_pair):
                        den_t = den_p.tile([1, 512], FP16, name="den")
                        if use_act:
                            nc.scalar.activation(
                                den_t[:], ot[64:65, :],
                                mybir.ActivationFunctionType.Copy,
                            )
                        else:
                            nc.vector.tensor_copy(den_t[:], ot[64:65, :])
                        nc.tensor.matmul(
                            bc_t[:64, :], ones_sb[:, :], den_t[:],
                            start=True, stop=True,
                        )
                        rc_t = rc_p.tile([64, 512], FP32, name="rc")
                        nc.vector.reciprocal_approx_fast(rc_t[:], bc_t[:64, :])
                        nc.vector.tensor_mul(
                            osb[mp][hl * 64 : (hl + 1) * 64,
                                    np_ * 512 : (np_ + 1) * 512],
                            ot[:64, :],
                            rc_t[:],
                        )

                # per-step feed budget in MM units (chunk pops overdraw by
                # design; the while-safeguards below guarantee deadlines)
                budgets = [6, 6, 5, 4, 3, 3, 2, 2]
                for it in range(8):
                    m, n = iters[it]
                    if it == 5:
                        # norm(3,0) was emitted during iter 4 -> the n=0
                        # half of osb is fully normalized in program order
                        for mo in range(4):
                            for no in range(2):
                                feed.append(("f", mo, no))
                    # safeguard: the iter's first scores need qt[m][n]
                    while feed and not qdone[m][n]:
                        pump(6)
                    ot_cur = (
                        otp.tile([128, 512], FP32, name="ot"),
                        otp.tile([128, 512], FP32, name="ot"),
                    )
                    p2_cur = []

                    def emit_score(c):
                        # one row-packed score pair into a fresh st tile
                        cc, co = c // 4, (c % 4) * 128
                        while feed and not kdone[m][cc]:
                            pump(6)
                        st = stp.tile([128, 1024], FP32, name="st")
                        _mm(
                            nc,
                            st[:, 0:512],
                            kt_sb[m][cc][0:64, co : co + 128],
                            qt_sb[m][n][0:64, :],
                            start=True,
                            stop=True,
                        )
                        _mm(
                            nc,
                            st[:, 512:1024],
                            kt_sb[m][cc][64:128, co : co + 128],
                            qt_sb[m][n][64:128, :],
                            start=True,
                            stop=True,
                        )
                        return st

                    # Steps emitted in PAIRS: the two score pairs go
                    # back-to-back so the second pair's LDWEIGHTS (rows
                    # 0:64) pulls ahead under the first pair's in-flight
                    # row-disjoint matmuls instead of waiting behind a
                    # full-row PV/feed matmul.
                    for c2 in range(8):
                        ca, cb = 2 * c2, 2 * c2 + 1
                        st_a = emit_score(ca)
                        st_b = emit_score(cb)
                        for c, st in ((ca, st_a), (cb, st_b)):
                            p2t = p2_p.tile([128, 1024], FP16, name="p2")
                            nc.scalar.activation(
                                p2t[:],
                                st[:],
                                mybir.ActivationFunctionType.Exp,
                                bias=0.0,
                                scale=SCALE,
                            )
                            p2_cur.append(p2t)
                        # lag-1 PV of the previous iteration
                        if p2_prev is not None:
                            mp, np_ = iters[it - 1]
                            for c in (ca, cb):
                                while feed and not vadone[c]:
                                    pump(6)
                                emit_pv(ot_prev, mp, c, p2_prev[c])
                            if cb == 15:
                                emit_norm(ot_prev, mp, np_)
                        pump(2 * budgets[it])
                    p2_prev, ot_prev = p2_cur, ot_cur

                # ---- Tail: PV of the last iter, then out-proj second half.
                # The scores/exps are done, so the stp banks host FOUR
                # pre-started psy accumulators: their kt4=0..2 partial sums
                # (osb[0..2] normalized during iters 5-7) run concurrently
                # with the final normalize chain; only each chunk's kt4=3
                # matmul waits for norm(3,1).
                mp, np_ = iters[7]
                for c in range(16):
                    emit_pv(ot_prev, mp, c, p2_prev[c])
                pre = []  # (psy_ap, mo, no)
                st_t1 = stp.tile([128, 1024], FP32, name="st")
                st_t2 = stp.tile([128, 1024], FP32, name="st")
                for i, (mo, no) in enumerate(
                    [(4, 0), (4, 1), (5, 0), (5, 1)]
                ):
                    t = st_t1 if i < 2 else st_t2
                    psy = t[:, (i % 2) * 512 : (i % 2 + 1) * 512]
                    pre.append((psy, mo, no))
                    for kt4 in range(3):
                        _mm(
                            nc,
                            psy,
                            osb[kt4][:, mo * 128 : (mo + 1) * 128],
                            wo_sb[:, kt4, no * 512 : (no + 1) * 512],
                            start=(kt4 == 0),
                            stop=False,
                        )
                emit_norm(ot_prev, mp, np_, use_act=True)
                for i, (psy, mo, no) in enumerate(pre):
                    _mm(
                        nc,
                        psy,
                        osb[3][:, mo * 128 : (mo + 1) * 128],
                        wo_sb[:, 3, no * 512 : (no + 1) * 512],
                        start=False,
                        stop=True,
                    )
                    ys = ys_p.tile([128, 512], FP32, name="ys")
                    if no == 0:
                        nc.scalar.activation(
                            ys[:], psy,
                            mybir.ActivationFunctionType.Copy,
                        )
                    else:
                        nc.vector.tensor_copy(ys[:], psy)
                    (nc.scalar if no == 1 else nc.sync).dma_start(
                        out[mo * 128 : (mo + 1) * 128, no * 512 : (no + 1) * 512],
                        ys[:],
                    )
                for mo in range(6, 8):
                    for no in range(2):
                        feed.append(("f", mo, no))
                pump(10**9)

    nc.finalize()
    return nc


def kernel(**inputs):
    global LAST_RESULT
    arrs = {k: np.asarray(v, dtype=np.float32) for k, v in inputs.items()}
    query, key, value = arrs["query"], arrs["key"], arrs["value"]
    Wq, bq_, Wk, bk_ = arrs["Wq"], arrs["bq"], arrs["Wk"], arrs["bk"]
    Wv, bv_, Wo, bo_ = arrs["Wv"], arrs["bv"], arrs["Wo"], arrs["bo"]

    nc = build_program()

    qTb = [np.ascontiguousarray(query[b].T.astype(np.float16)) for b in range(B)]
    kTb = [np.ascontiguousarray(key[b].T.astype(np.float16)) for b in range(B)]
    vTb = [np.ascontiguousarray(value[b].T.astype(np.float16)) for b in range(B)]

    per_group = []
    for g in range(2):
        gs = slice(g * GO, (g + 1) * GO)
        wq_m = np.ascontiguousarray(Wq[gs, :].T.astype(np.float16))
        wk_m = np.ascontiguousarray(Wk[gs, :].T.astype(np.float16))
        wv_m = np.ascontiguousarray(Wv[gs, :].T.astype(np.float16))
        vb_row = bv_[gs].astype(np.float32)  # head-major [8*64]
        vbias_m = np.ascontiguousarray(np.tile(vb_row, (128, 1)).astype(np.float32))
        wo_m = np.ascontiguousarray(Wo[:, gs].T.astype(np.float16))
        bq_m = np.ascontiguousarray(bq_[gs].reshape(4, 128).T)
        bk_m = np.ascontiguousarray(bk_[gs].reshape(4, 128).T)
        per_group.append(
            {
                "wq": wq_m,
                "wk": wk_m,
                "wv": wv_m,
                "wo": wo_m,
                "vbias": vbias_m,
                "bq": bq_m,
                "bk": bk_m,
            }
        )

    in_maps = []
    for c in range(8):
        b, g = c // 2, c % 2
        m = {"qT": qTb[b], "kT": kTb[b], "vT": vTb[b]}
        m.update(per_group[g])
        in_maps.append(m)

    res = run_bass_kernel_spmd(
        nc, in_maps, list(range(8)), trace=TRACE, **(TRACE_KWARGS if TRACE else {})
    )
    LAST_RESULT = res

    outs = res.results
    Y = np.empty((B, NQ, E), np.float32)
    for b in range(B):
        Y[b] = outs[2 * b]["out"] + outs[2 * b + 1]["out"] + bo_[None, :]
    return Y
